# revision 7
# baseline (speedup 1.0000x reference)
"""Trainium2 Bass kernel for nn_BQuantConv1d_simple.

Math: out[t, n] = sum_k (x2 @ binary[k])[t, n] * scale[k, 0, n] + bias[n]
with x2 = x.reshape(T, M).  scale has no m/t dependence, so it folds:

    W[m, n] = sum_k binary[k, m, n] * scale[k, 0, n]
    out     = x2 @ W + bias

which cuts the tensor-engine work 8x versus the unfolded form.

Two SPMD launches across the 8 NeuronCores:

  L1 (bit-sharded fold): core c computes Wc^T = (binary[c] * scale[c])^T.
     binary is +/-1 so it ships losslessly as fp8e4m3 (1 MB instead of 2);
     the transposed [n, m] layout makes scale a per-PARTITION scalar, so the
     multiplies split across the Scalar (ACT) and Vector (DVE) engines in
     parallel.  The host sums the 8 partials in fp32 — the standard unshard
     step for a reduction-sharded computation.

  L2 (token-sharded matmul): core c computes out[tc] = x2[tc] @ W + bias on
     the tensor engine in fp16 (fp32 PSUM accumulation).  x is fed
     pre-transposed (m on partitions) since the PE contracts the partition
     axis of both operands.
"""

import numpy as np

import concourse.bass as bass
import concourse.mybir as mybir
import concourse.tile as tile
import concourse.tile_sem_assignment as _tsa
from concourse.bass_utils import run_bass_kernel_spmd

# Rotating HWDGE completion semaphores over fewer lanes shrinks the
# kernel-tail dma_reset/sem_clear chain (inside the measured window) and
# the number of multi-wait legalizer NoOps; waits are value-based so
# correctness is unchanged.
_HWDGE_LANES = {"l1": 2, "l2": 4}

F8 = mybir.dt.float8e4
F16 = mybir.dt.float16
F32 = mybir.dt.float32

K, M, N = 8, 1024, 1024
B_, S_ = 4, 2048
T = B_ * S_            # 8192 tokens
NCORES = 8
TPC = T // NCORES      # 1024 tokens per core
P = 128                # partitions

_nc_cache = {}


def _legalize_sync_waits(nc):
    """This container's walrus build only accepts ONE sync-wait command per
    instruction (setupSyncWait in CoreV3GenImpl rejects more).  Tile emits
    up to 4.  Split the extras into single-wait NoOps placed immediately
    before the instruction on the same engine — the sequencer executes them
    in order, so the semantics are identical."""
    cnt = 0
    for fn in nc.m.functions:
        for blk in fn.blocks:
            insts = list(blk.instructions)
            out = []
            for inst in insts:
                si = inst.sync_info
                if si is not None and si.on_wait and len(si.on_wait) > 1:
                    waits = list(si.on_wait)
                    for w in waits[:-1]:
                        nop = mybir.InstNoOp(
                            name=f"legalize_wait_{cnt}", ins=[], outs=[])
                        cnt += 1
                        nop.engine = inst.engine
                        nop.sync_info = mybir.SyncInfo(on_wait=[w], on_update=[])
                        out.append(nop)
                    inst.sync_info = mybir.SyncInfo(
                        on_wait=[waits[-1]], on_update=list(si.on_update or []))
                out.append(inst)
            blk.instructions = out
    return nc


def _build_l1():
    """Per-core (n-slice c): w_part[n, m] = sum_k binary[k].T[n, m]*scale[k, n]
    for the 128 n-rows this core owns — the FULL fold for its slice, so the
    host only concatenates (no reduction, 0.25 MB output instead of 2 MB).

    Transposed layout puts n on partitions, so scale[k, n] is a per-partition
    scalar and the k-accumulation is a chain of fused multiply-adds on the
    DVE: w = (b_k * s_k) + w via scalar_tensor_tensor.  binary ships
    partition-major as fp8e4m3 (+/-1 is exact): [128 n][8 k][1024 m] puts
    8 KB contiguous per partition; it loads in two 4 KB half-chunks so the
    chain starts after 0.5 MB."""
    nc = bass.Bass("TRN2", num_devices=NCORES, enable_asserts=False)
    b_in = nc.dram_tensor("b_in", [P, K, M], F8, kind="ExternalInput")
    s_in = nc.dram_tensor("s_in", [P, K], F32, kind="ExternalInput")
    w_out = nc.dram_tensor("w_part", [P, M], F16, kind="ExternalOutput")

    with tile.TileContext(nc) as tc:
        with tc.tile_pool(name="work", bufs=1) as pool:
            # All loads first: the SP sequencer is in-order, so a store that
            # waits on compute must not sit ahead of an independent load.
            s_sb = pool.tile([P, K], F32, tag="s")
            nc.sync.dma_start(s_sb[:], s_in[:])
            b_sbs = []
            for ci in range(2):
                b_sb = pool.tile([P, K // 2, M], F8, tag=f"b{ci}", name=f"b{ci}")
                nc.sync.dma_start(b_sb[:], b_in[:, ci * (K // 2):(ci + 1) * (K // 2), :])
                b_sbs.append(b_sb)
            # Ping-pong accumulators keep each fused op's output distinct
            # from its in1 so the tile dataflow stays acyclic.
            acc = [pool.tile([P, M], F16, tag=f"w{i}", name=f"w{i}")
                   for i in range(2)]
            nc.vector.tensor_scalar_mul(
                acc[0][:], b_sbs[0][:, 0, :], s_sb[:, 0:1])
            for k in range(1, K):
                nc.vector.scalar_tensor_tensor(
                    acc[k % 2][:],
                    b_sbs[k // (K // 2)][:, k % (K // 2), :],
                    s_sb[:, k:k + 1],
                    acc[(k - 1) % 2][:],
                    mybir.AluOpType.mult,
                    mybir.AluOpType.add,
                )
            nc.sync.dma_start(w_out[:], acc[(K - 1) % 2][:])
    return nc


def _build_l2():
    """Per-core: out = x2[tc] @ W + bias (token shard).

    W and xT are fed as ONE fused input wx [M, N + TPC] so each m-block
    arrives in a single 0.5 MB DMA.  Loop is mb-outer over 4 token-tiles
    at a time (8 PSUM banks = 4 tt x 2 nb accumulation groups), so the
    matmul stream starts as soon as wx[0] lands and is never load-starved."""
    nc = bass.Bass("TRN2", num_devices=NCORES, enable_asserts=False)
    wx_in = nc.dram_tensor("wx_in", [M, N + TPC], F16, kind="ExternalInput")
    bias_in = nc.dram_tensor("bias_in", [P, N], F16, kind="ExternalInput")
    # fp16 output store (host upcasts): halves store wire and the tail
    # transfer; the fp32 PSUM accumulation is unaffected and the fp16
    # rounding (2^-11) is below the fp16-input noise already present.
    out = nc.dram_tensor("out", [TPC, N], F16, kind="ExternalOutput")

    MB = M // P        # 8 contraction tiles
    TT = TPC // P      # 8 token tiles
    NBW = 512          # one PSUM bank of fp32
    NB = N // NBW      # 2 n blocks
    TG = 4             # token-tiles processed per group (TG*NB = 8 banks)
    NWARM = 5          # ~2.5us of PE ramp at the cold clocks, bridging to wx0

    with tile.TileContext(nc) as tc:
        with (
            tc.tile_pool(name="const", bufs=1) as cpool,
            tc.tile_pool(name="psum", bufs=1, space=bass.MemorySpace.PSUM) as ppool,
            tc.tile_pool(name="out", bufs=4) as opool,
        ):
            # PE warmup: the HAM clock gate needs ~3us of sustained PE
            # activity before it lifts the 1.2GHz -> 2.4GHz throttle.  Run
            # dummy matmuls on a zeroed scratch tile while the first wx
            # DMA is in flight; the first REAL matmuls then continue the
            # ramp, so only the bridge to wx0 is spent on dummies.  The
            # memset runs on the DVE (fast) so warmup starts immediately.
            # The warm psum uses the bank tag whose first real use comes
            # latest, so warmup never delays a real accumulation group.
            warm_sb = cpool.tile([P, NBW], F16, tag="warm")
            nc.vector.memset(warm_sb[:], 0.0)
            warm_ps = ppool.tile([P, NBW], F32, tag=f"ps_{TG-1}_{NB-1}",
                                 name="warm_ps")
            # One accumulation group: chained start=False matmuls run
            # back-to-back on the PE with no inter-matmul semaphore round
            # trips, so the ramp is continuous.
            for i in range(NWARM):
                nc.tensor.matmul(
                    warm_ps[:], warm_sb[:, :P], warm_sb[:],
                    start=(i == 0), stop=(i == NWARM - 1),
                )

            # wx loads first (the wire serializes from the first transfer,
            # so the matmul-critical loads must lead); bias is consumed
            # ~10us later and rides at the back of the queue.
            wx_sb = []
            for mb in range(MB):
                wx_t = cpool.tile([P, N + TPC], F16, tag=f"wx{mb}",
                                  name=f"wx{mb}")
                nc.sync.dma_start(wx_t[:], wx_in[mb * P:(mb + 1) * P, :])
                wx_sb.append(wx_t)
            bias_sb = cpool.tile([P, N], F16, tag="bias")
            nc.sync.dma_start(bias_sb[:], bias_in[:])

            # First group: 4 token-tiles (8 banks) so early matmul demand
            # stays below the streaming-load rate.  Then single-tile groups
            # (2 banks each) so the final bias-add/store tail is short.
            groups = [list(range(TG))] + [[tt] for tt in range(TG, TT)]
            last_tt = TT - 1
            for grp in groups:
                psums = {}
                for tt in grp:
                    for nb in range(NB):
                        psums[(tt, nb)] = ppool.tile(
                            [P, NBW], F32, tag=f"ps_{tt % TG}_{nb}",
                            name=f"ps{tt}_{nb}")
                if grp == [last_tt]:
                    # nb-outer so nb=0 finishes a full mb-loop early; its
                    # bias-add/store runs under nb=1's matmuls and the tail
                    # after the last matmul is a single add+store.
                    for nb in range(NB):
                        for mb in range(MB):
                            nc.tensor.matmul(
                                psums[(last_tt, nb)][:],
                                wx_sb[mb][:, N + last_tt * P:N + (last_tt + 1) * P],
                                wx_sb[mb][:, nb * NBW:(nb + 1) * NBW],
                                start=(mb == 0),
                                stop=(mb == MB - 1),
                            )
                        nsl = slice(nb * NBW, (nb + 1) * NBW)
                        o_t = opool.tile([P, NBW], F16, tag="olast",
                                         name=f"o{last_tt}_{nb}")
                        nc.vector.tensor_add(
                            o_t[:], psums[(last_tt, nb)][:], bias_sb[:, nsl])
                        nc.sync.dma_start(
                            out[last_tt * P:(last_tt + 1) * P, nsl], o_t[:])
                    continue
                for mb in range(MB):
                    for tt in grp:
                        lhsT = wx_sb[mb][:, N + tt * P:N + (tt + 1) * P]
                        for nb in range(NB):
                            nc.tensor.matmul(
                                psums[(tt, nb)][:],
                                lhsT,
                                wx_sb[mb][:, nb * NBW:(nb + 1) * NBW],
                                start=(mb == 0),
                                stop=(mb == MB - 1),
                            )
                for tt in grp:
                    # One [128, 1024] store per token tile: 2 KB rows DMA
                    # twice as efficiently as the 1 KB rows of per-bank
                    # stores.
                    o_t = opool.tile([P, N], F16, tag="o", name=f"o{tt}")
                    for nb in range(NB):
                        nsl = slice(nb * NBW, (nb + 1) * NBW)
                        nc.vector.tensor_add(
                            o_t[:, nsl], psums[(tt, nb)][:], bias_sb[:, nsl])
                    nc.sync.dma_start(out[tt * P:(tt + 1) * P, :], o_t[:])
    return nc


def _strip_dead_const_memsets(nc):
    """Bass unconditionally emits 4 memsets for its const-AP tiles; when
    nothing reads them they only lengthen the pre-block rendezvous on
    GpSimd.  Drop memsets whose const-* destination has no reader."""
    readers = set()
    memsets = []
    for fn in nc.m.functions:
        for blk in fn.blocks:
            for inst in blk.instructions:
                for ap in (inst.ins or []):
                    mr = getattr(ap, "memref", None)
                    if mr:
                        readers.add(mr)
                if type(inst).__name__ == "InstMemset":
                    outs = inst.outs or []
                    mr = getattr(outs[0], "memref", None) if outs else None
                    if mr and mr.startswith("const-"):
                        memsets.append(mr)
    dead = {mr for mr in memsets if mr not in readers}
    if dead:
        for fn in nc.m.functions:
            for blk in fn.blocks:
                blk.instructions = [
                    inst for inst in blk.instructions
                    if not (type(inst).__name__ == "InstMemset"
                            and (inst.outs or [])
                            and getattr(inst.outs[0], "memref", "") in dead)
                ]
    return nc


def _trim_initial_barrier(nc):
    """Bass's __init__ ends with an all-engine barrier that orders the init
    sem-clears and const-AP memsets before the body.  In this flow the
    clears aren't emitted (no BIR lowering) and the dead memsets are
    stripped, so the barrier only serializes per-engine preambles that
    need no cross-engine ordering — and it delays the first DMA issue by
    ~1us inside the measured window.  Drop its Drain+EventSemaphore pairs
    from the init block."""
    blk = nc.m.functions[0].blocks[0]
    blk.instructions = [
        inst for inst in blk.instructions
        if not (
            (type(inst).__name__ == "InstEventSemaphore"
             and str(getattr(inst, "name", "")).startswith("barrier_"))
            or type(inst).__name__ == "InstDrain"
        )
    ]
    return nc


def _trim_final_barrier(nc):
    """bass.reset() ends the kernel with [barrier, sem/dma resets, barrier].
    The second all-engine barrier only isolates the resets from a
    re-execution of the same loaded NEFF, which this flow never does (each
    call builds a fresh executable), and the Pool engine still halts after
    its resets, so NEFF completion already orders them.  Drop the trailing
    drain+event-semaphore round (~3us inside the measured window)."""
    for fn in nc.m.functions:
        if not fn.blocks:
            continue
        blk = fn.blocks[-1]
        insts = list(blk.instructions)
        while insts and type(insts[-1]).__name__ in (
                "InstDrain", "InstEventSemaphore", "InstNoOp"):
            insts.pop()
        blk.instructions = insts
    return nc


def _get_nc(name):
    if name not in _nc_cache:
        prev = _tsa.NUM_HWDGE_SEMS
        _tsa.NUM_HWDGE_SEMS = _HWDGE_LANES[name]
        try:
            nc = {"l1": _build_l1, "l2": _build_l2}[name]()
        finally:
            _tsa.NUM_HWDGE_SEMS = prev
        _nc_cache[name] = _trim_final_barrier(
            _legalize_sync_waits(
                _trim_initial_barrier(_strip_dead_const_memsets(nc))))
    return _nc_cache[name]


def run_sharded(x, binary, scale, bias, trace=False):
    """Returns (out_full, [l1_results, l2_results])."""
    x = np.asarray(x, dtype=np.float32)
    binary = np.asarray(binary, dtype=np.float32)
    scale = np.asarray(scale, dtype=np.float32)
    bias = np.asarray(bias, dtype=np.float32)

    core_ids = list(range(NCORES))
    f8np = mybir.dt.np(F8)

    # ---- L1: n-sliced scale fold (transposed, fp8 signs) ----------------
    in_maps1 = []
    for c in range(NCORES):
        ns = slice(c * P, (c + 1) * P)
        in_maps1.append({
            # [128 n, 8 k, 1024 m]; +/-1 is exact in fp8e4m3
            "b_in": np.ascontiguousarray(
                binary[:, :, ns].transpose(2, 0, 1)).astype(f8np),
            "s_in": np.ascontiguousarray(scale[:, 0, ns].T),   # [128 n, 8 k]
        })
    r1 = run_bass_kernel_spmd(_get_nc("l1"), in_maps1, core_ids, trace=trace)

    wT16 = np.concatenate(
        [r1.results[c]["w_part"] for c in range(NCORES)], axis=0)  # [N, M]
    w16 = np.ascontiguousarray(wT16.T)

    # ---- L2: token-sharded matmul ---------------------------------------
    x2 = x.reshape(T, M)
    bias_b = np.ascontiguousarray(
        np.broadcast_to(bias, (P, N))).astype(np.float16)
    in_maps2 = []
    for c in range(NCORES):
        wx = np.empty((M, N + TPC), dtype=np.float16)   # [W | xT] fused
        wx[:, :N] = w16
        wx[:, N:] = x2[c * TPC:(c + 1) * TPC].T
        in_maps2.append({"wx_in": wx, "bias_in": bias_b})
    r2 = run_bass_kernel_spmd(_get_nc("l2"), in_maps2, core_ids, trace=trace)

    out = np.concatenate(
        [r2.results[c]["out"] for c in range(NCORES)], axis=0).astype(np.float32)
    return out.reshape(B_, S_, N), [r1, r2]


def kernel(x, binary, scale, bias):
    out, _ = run_sharded(x, binary, scale, bias, trace=False)
    return out
